# revision 24
# baseline (speedup 1.0000x reference)
"""Trainium2 Bass kernel for a dense transformer block (attention + ReLU FFN).

Reference computation (B=4, T=2048, C=1024, H=16, D=64):
    q,k,v = per-head projections of x;  causal softmax(q k^T / sqrt(C)) v;
    concat heads;  y = relu(out @ Wf.T + bf)

Sharding over 8 NeuronCores: core (2b+p) handles batch b with heads
[8p, 8p+8).  Attention runs causally over the full T on each core.  Pair
AllGathers (cores 2b/2b+1) share the attention outputs, and each core
runs the FFN for all 2048 tokens over its own half of the output
channels (the channel split is carried entirely by per-core input data -
every core executes an identical NEFF).

Layouts: scores are computed transposed ([s, t], keys on partitions) so
the exp() output feeds the AV matmul directly; V carries an appended
ones-column so row 64 of the AV accumulator is the softmax denominator;
causal masking is a -1e4 rank-128 matmul accumulated into the diagonal
score tile before exp. Compute dtype bf16 with fp32 PSUM accumulation.

Scheduling: a 48-matmul warmup burst on the (tiny, loaded-first) mask
constants heats the PE HAM clock-gate while the input DMAs stream;
projection/FFN matmul chunks are drained from filler queues inside the
attention j-loops so the PE never idles long enough to re-throttle; the
AllGathers run pair-wise and are emitted as soon as their two heads are
staged (the z-broadcast DMAs ride the Sync queue so the GpSimd queue
holds only collective triggers); the second-half FFN accumulates in
three phases so only the ci{3,7} matmuls depend on the last AllGather.
"""

import os
import sys

from collections import deque

import numpy as np
import ml_dtypes

for _p in ("/opt/trn_rl_repo", "/root/.axon_site/_ro/trn_rl_repo"):
    if os.path.isdir(_p) and _p not in sys.path:
        sys.path.append(_p)

B, T, C, H, D = 4, 2048, 1024, 16, 64
P = 128           # partitions
NCT = C // P      # 8 c-tiles
NTT = T // P      # 16 s/t-tiles
HPC = H // 2      # 8 heads per core
THALF = T // 2    # tokens per AllGather half
COH = C // 2      # output channels per core in the FFN
SCALE = float(C) ** -0.5
WARM_N = 48       # PE warmup matmuls (heats the HAM clock gate)

bf16 = ml_dtypes.bfloat16

_CACHE = {}


def build_nc():
    import concourse.bass as bass
    import concourse.tile as tile
    from concourse import bacc, mybir

    f32 = mybir.dt.float32
    b16 = mybir.dt.bfloat16
    EXP = mybir.ActivationFunctionType.Exp

    nc = bacc.Bacc("TRN2", target_bir_lowering=False, debug=False, num_devices=8)

    xT = nc.dram_tensor("xT", [C, T], b16, kind="ExternalInput").ap()
    wq = nc.dram_tensor("wq", [C, HPC * D], b16, kind="ExternalInput").ap()
    wk = nc.dram_tensor("wk", [C, HPC * D], b16, kind="ExternalInput").ap()
    wv = nc.dram_tensor("wv", [C, HPC * D], b16, kind="ExternalInput").ap()
    wfT = nc.dram_tensor("wfT", [C, COH], b16, kind="ExternalInput").ap()
    mey = nc.dram_tensor("mey", [P, P], b16, kind="ExternalInput").ap()
    mls_ = nc.dram_tensor("mls", [P, P], b16, kind="ExternalInput").ap()
    biasb = nc.dram_tensor("biasb", [P, COH], f32, kind="ExternalInput").ap()
    y = nc.dram_tensor("y", [T, COH], b16, kind="ExternalOutput").ap()
    # warmup sink: ExternalOutput so the warmup matmuls can't be DCE'd
    wsink = nc.dram_tensor("wsink", [P, P], f32, kind="ExternalOutput").ap()

    with tile.TileContext(nc) as tc, \
            tc.tile_pool(name="consts", bufs=1) as consts, \
            tc.tile_pool(name="dram", bufs=1, space="DRAM") as dram, \
            tc.tile_pool(name="sc_ps", bufs=2, space="PSUM") as sc_pool, \
            tc.tile_pool(name="av_ps", bufs=1, space="PSUM") as av_pool, \
            tc.tile_pool(name="flex_ps", bufs=2, space="PSUM") as flex_pool, \
            tc.tile_pool(name="wt", bufs=3) as wt_pool, \
            tc.tile_pool(name="norm", bufs=2) as norm_pool, \
            tc.tile_pool(name="yout", bufs=3) as y_pool:

        xT_sb = consts.tile([P, NCT, T], b16)
        wq_sb = consts.tile([P, NCT, HPC * D], b16)
        wk_sb = consts.tile([P, NCT, HPC * D], b16)
        wv_sb = consts.tile([P, NCT, HPC * D], b16)
        wfT_sb = consts.tile([P, NCT, COH], b16)
        mey_sb = consts.tile([P, P], b16)
        mls_sb = consts.tile([P, P], b16)
        biasb_sb = consts.tile([P, COH], f32)
        qT_sb = consts.tile([P, HPC // 2, T], b16)
        kT_sb = consts.tile([P, HPC // 2, T], b16)
        v_sb = consts.tile([P, NTT, HPC, D + 1], b16)
        ccout_sb = consts.tile([P, 2, NCT, THALF], b16)

        cc_in = [dram.tile([HPC * D, THALF], b16, name=f"cc_in{i}") for i in (0, 1)]
        cc_out = [[dram.tile([C // 4, THALF], b16, name=f"cc_out{th}_{p}")
                   for p in range(4)] for th in (0, 1)]
        cc_out1s = [dram.tile([P, THALF], b16, name=f"cc_out1s_{h}")
                    for h in (6, 7)]

        # ---- constant loads: per-ct pieces (parallel DMA sub-queues),
        # weights on the Sync HWDGE group and x on the Scalar HWDGE group so
        # the preamble isn't serialized on a single engine's DMA issue.
        nc.sync.dma_start(out=mey_sb, in_=mey)
        nc.sync.dma_start(out=mls_sb, in_=mls_)
        xT_r = xT.rearrange("(ct p) t -> ct p t", p=P)
        wq_r = wq.rearrange("(ct p) m -> ct p m", p=P)
        wk_r = wk.rearrange("(ct p) m -> ct p m", p=P)
        wv_r = wv.rearrange("(ct p) m -> ct p m", p=P)
        for ct in range(NCT):
            nc.sync.dma_start(out=wq_sb[:, ct, :], in_=wq_r[ct])
            nc.scalar.dma_start(out=xT_sb[:, ct, 0:512], in_=xT_r[ct][:, 0:512])
        for ct in range(NCT):
            nc.sync.dma_start(out=wk_sb[:, ct, :], in_=wk_r[ct])
            nc.scalar.dma_start(out=xT_sb[:, ct, 512:THALF],
                                in_=xT_r[ct][:, 512:THALF])
        for ct in range(NCT):
            nc.sync.dma_start(out=wv_sb[:, ct, :], in_=wv_r[ct])
        for ct in range(NCT):
            nc.scalar.dma_start(out=xT_sb[:, ct, THALF:THALF + 512],
                                in_=xT_r[ct][:, THALF:THALF + 512])
            nc.scalar.dma_start(out=xT_sb[:, ct, THALF + 512:T],
                                in_=xT_r[ct][:, THALF + 512:T])
        wfT_r = wfT.rearrange("(ct p) co -> ct p co", p=P)
        for ct in range(NCT):
            nc.sync.dma_start(out=wfT_sb[:, ct, :], in_=wfT_r[ct])
        nc.sync.dma_start(out=biasb_sb, in_=biasb)
        # ones in column 0 of v: row 0 of the AV accumulator is then the
        # softmax denominator, already at PSUM partition 0 where the
        # custom-DVE reciprocal needs it (no ScalarE extraction copy).
        nc.vector.memset(v_sb[:, :, :, 0:1], 1.0)

        # ---- PE warmup: dense matmul burst on the mask constants while the
        # big input DMAs stream; keeps the HAM gate at 8/8 for the real work.
        with nc.named_scope("warmup"):
            wps = flex_pool.tile([P, P], f32, tag="flex", name="warmps")
            for i in range(WARM_N):
                nc.tensor.matmul(wps, lhsT=mey_sb, rhs=mls_sb,
                                 start=(i == 0), stop=(i == WARM_N - 1))
            wsb = y_pool.tile([P, P], f32, tag="y", name="warmsb")
            nc.vector.tensor_copy(out=wsb, in_=wps)
            nc.sync.dma_start(out=wsink, in_=wsb)

        # ---- emission helpers ----------------------------------------------
        def v_proj(st):
          with nc.named_scope("vproj"):
            ps = flex_pool.tile([P, 512], f32, tag="flex", name=f"vps{st}")
            for ct in range(NCT):
                nc.tensor.matmul(
                    ps, lhsT=xT_sb[:, ct, P * st:P * (st + 1)],
                    rhs=wv_sb[:, ct, :],
                    start=(ct == 0), stop=(ct == NCT - 1))
            nc.vector.tensor_copy(out=v_sb[:, st, :, 1:D + 1],
                                  in_=ps.rearrange("p (h d) -> p h d", d=D))

        def qk_chunk(hp, i):
          with nc.named_scope("qkproj"):
            dst, w_t = ((qT_sb, wq_sb), (kT_sb, wk_sb))[i // 4]
            g = i % 4
            ps = flex_pool.tile([P, 512], f32, tag="flex", name=f"qkps{hp}_{i}")
            for ct in range(NCT):
                nc.tensor.matmul(
                    ps, lhsT=w_t[:, ct, hp * P:(hp + 1) * P],
                    rhs=xT_sb[:, ct, 512 * g:512 * (g + 1)],
                    start=(ct == 0), stop=(ct == NCT - 1))
            nc.vector.tensor_copy(out=dst[:, hp, 512 * g:512 * (g + 1)], in_=ps)

        def attn_unit(h, th, mid=None):
          with nc.named_scope(f"attn{th}_{h}"):
            hp, qh = divmod(h, 2)
            base = 64 * qh
            t0 = THALF * th
            av = av_pool.tile([P, THALF], f32, tag="av", name=f"av{h}_{th}")
            jmax = 8 * th + 8
            last_j = {0: 8 * th + 3, 1: jmax - 1}
            pend = None  # (j, pieces, wt) awaiting its AV emission

            def emit_av(ent):
                j, pieces, wt = ent
                for (o, e) in pieces:
                    region = 0 if o < 512 else 1
                    nc.tensor.matmul(
                        av[0:D + 1, o:e], lhsT=v_sb[:, j, h, :], rhs=wt[:, o:e],
                        start=(j == 0), stop=(j == last_j[region]))

            for j in range(jmax):
                off = max(0, P * j - t0)
                diag = P * j >= t0
                pieces = [(off, 512), (512, 1024)] if off < 512 \
                    else [(off, 1024)]
                sc = sc_pool.tile([P, THALF], f32, tag="sc", name=f"sc{h}_{th}_{j}")
                for pi, (o, e) in enumerate(pieces):
                    nc.tensor.matmul(
                        sc[:, o:e],
                        lhsT=kT_sb[base:base + 64, hp, P * j:P * (j + 1)],
                        rhs=qT_sb[base:base + 64, hp, t0 + o:t0 + e],
                        start=True, stop=not (diag and pi == 0))
                if diag:  # causal mask: accumulate -1e4 below the diagonal
                    nc.tensor.matmul(
                        sc[:, off:off + P], lhsT=mey_sb, rhs=mls_sb,
                        start=False, stop=True)
                wt = wt_pool.tile([P, THALF], b16, tag="wt", name=f"wt{h}_{th}_{j}")
                nc.scalar.activation(out=wt[:, off:THALF], in_=sc[:, off:THALF],
                                     func=EXP, scale=SCALE)
                if pend is not None:
                    emit_av(pend)
                if mid is not None and j in mid:
                    mid[j]()
                pend = (j, pieces, wt)
            emit_av(pend)
            # av row 0 is the softmax denominator (ones-column of v): the
            # custom-DVE reciprocal reads it straight from PSUM partition 0.
            zr = norm_pool.tile([1, THALF], f32, tag="zr", name=f"zrr{h}_{th}")
            nc.vector.reciprocal_approx_fast(out=zr, in_=av[0:1, 0:THALF])
            # evacuate the accumulator in one fast copy (frees the PSUM
            # slot for the next unit), then normalize u/Z off-path from SBUF
            avc = norm_pool.tile([D + 1, THALF], f32, tag="avc", name=f"avc{h}_{th}")
            nc.vector.tensor_copy(out=avc, in_=av[0:D + 1, 0:THALF])

            def fin():
                # broadcast 1/Z + stage, deferred into the NEXT unit so the
                # Scalar-queue dma_starts never stall on this unit's DVE
                # chain (and GpSimd keeps only collective triggers)
                zb = norm_pool.tile([D + 1, THALF], f32, tag="zb",
                                    name=f"zb{h}_{th}")
                zr_b = bass.AP(tensor=zr.tensor, offset=zr.offset,
                               ap=[list(zr.ap[0]), [0, D + 1], [1, THALF]])
                nc.scalar.dma_start(out=zb, in_=zr_b)
                # multiply rows 0:65 (partition-0-aligned for the DVE); row 0
                # is Z * 1/Z and is simply not staged
                stage = norm_pool.tile([D + 1, THALF], b16, tag="stage",
                                       name=f"st{h}_{th}")
                nc.vector.tensor_mul(out=stage, in0=avc, in1=zb)
                nc.scalar.dma_start(out=cc_in[th][64 * h:64 * (h + 1), :],
                                    in_=stage[1:D + 1, :])
            return fin

        RG = [[0, 1], [2, 3], [4, 5], [6, 7]]

        def allgather(th, p):
          # head pair {2p, 2p+1} of token-half th -> ci-tiles p (rank0) and
          # 4+p (rank1), each complete
          with nc.named_scope(f"ag{th}_{p}"):
            import concourse.mybir as mybir_mod
            nc.gpsimd.collective_compute(
                "AllGather", mybir_mod.AluOpType.bypass, replica_groups=RG,
                ins=[cc_in[th][128 * p:128 * (p + 1), :].opt()],
                outs=[cc_out[th][p].opt()])
            cc_r = cc_out[th][p].rearrange("(ci p2) t -> ci p2 t", p2=P)
            nc.sync.dma_start(out=ccout_sb[:, th, p, :], in_=cc_r[0])
            nc.sync.dma_start(out=ccout_sb[:, th, 4 + p, :], in_=cc_r[1])

        def allgather_single(h):
          # single head h of token-half 1 -> 64-row halves of ci-tiles h//2
          # (rank0) and 4 + h//2 (rank1); keeps the tail AllGather small
          with nc.named_scope(f"ag1s_{h}"):
            import concourse.mybir as mybir_mod
            nc.gpsimd.collective_compute(
                "AllGather", mybir_mod.AluOpType.bypass, replica_groups=RG,
                ins=[cc_in[1][64 * h:64 * (h + 1), :].opt()],
                outs=[cc_out1s[h - 6].opt()])
            r0 = 64 * (h % 2)
            nc.sync.dma_start(out=ccout_sb[r0:r0 + 64, 1, h // 2, :],
                              in_=cc_out1s[h - 6][0:64, :])
            nc.sync.dma_start(out=ccout_sb[r0:r0 + 64, 1, 4 + h // 2, :],
                              in_=cc_out1s[h - 6][64:128, :])

        def ffn_tile0(tt):
          # full single-pass FFN tile for token-half 0 (all AGs landed)
          with nc.named_scope("ffn"):
            ps = flex_pool.tile([P, COH], f32, tag="flex", name=f"fps{tt}")
            for k, ci in enumerate((0, 4, 1, 5, 2, 6, 3, 7)):
                nc.tensor.matmul(
                    ps, lhsT=ccout_sb[:, 0, ci, P * tt:P * (tt + 1)],
                    rhs=wfT_sb[:, ci, :],
                    start=(k == 0), stop=(k == NCT - 1))
            ysb = y_pool.tile([P, COH], b16, tag="y", name=f"y{tt}")
            nc.vector.tensor_add(out=ysb, in0=ps, in1=biasb_sb)
            nc.vector.tensor_scalar_max(ysb, ysb, 0.0)
            nc.sync.dma_start(out=y.rearrange("(tt p) co -> tt p co", p=P)[tt],
                              in_=ysb)

        # ---- filler queues: projection/FFN chunks drained into the
        # attention j-loops to keep TensorE dense (and the HAM gate warm).
        # QA runs during token-half 0 (everything th1 units need up front);
        # QB runs inside the th1 units, sized to their exp-paced slack.
        fillQA = deque()   # th0-phase fillers (input-DMA gated only)
        fillQB = deque()   # th1-phase fillers (deadline-checked per unit)
        for hp in (1, 2, 3):
            for i in (0, 4, 1, 5):
                fillQA.append(lambda hp=hp, i=i: qk_chunk(hp, i))
        for st in range(8, NTT):
            fillQA.append(lambda st=st: v_proj(st))
        for i in (2, 6, 3, 7):
            fillQA.append(lambda i=i: qk_chunk(0, i))
        for hp in (1, 2, 3):   # hp g23 needed before th1 unit 2*hp
            for i in (2, 6, 3, 7):
                fillQB.append(lambda hp=hp, i=i: qk_chunk(hp, i))
        for tt in range(8):    # th0 FFN tiles (gated on the th0 AllGathers)
            fillQB.append(lambda tt=tt: ffn_tile0(tt))

        def popA():
            if fillQA:
                fillQA.popleft()()

        def popB():
            if fillQA:
                fillQA.popleft()()
            elif fillQB:
                fillQB.popleft()()

        # ---- emission order --------------------------------------------------
        # Each unit's fin (zb/stage issue) runs at the NEXT unit's j=1, when
        # its inputs are long since ready; an AllGather must therefore also be
        # EMITTED after the fin of its second unit (Tile only tracks writers
        # that already exist), so it rides the next unit's j=2 slot.
        # upfront: q/k for head-pair 0 over tokens 0:1024, v tiles 0:4
        for i in (0, 4, 1, 5):
            qk_chunk(0, i)
        for st in range(4):
            v_proj(st)
        # token-half 0 attention; v st4-7 finish inside unit 0
        fin = attn_unit(0, 0, mid={1: lambda: v_proj(4), 2: lambda: v_proj(5),
                                   3: lambda: v_proj(6), 4: lambda: v_proj(7),
                                   5: popA, 7: popA})
        popA()
        pend_ag = None
        for h in range(1, HPC):
            mids = {1: fin, 3: popA, 5: popA}
            if pend_ag is not None:
                mids[2] = pend_ag
            fin = attn_unit(h, 0, mid=mids)
            if h % 2 == 1:
                pend_ag = (lambda p=h // 2: allgather(0, p))
            popA()
        while fillQA:  # all projection work must land before token-half 1
            popA()

        # token-half 1 attention: qk g23 chunks and th0 FFN tiles are spread
        # across the units to match their exp-paced slack
        fin = attn_unit(0, 1, mid={1: fin, 2: pend_ag,
                                   5: popB, 8: popB, 11: popB})
        pend_ag = None
        for h in range(1, HPC):
            mids = {1: fin, 6: popB, 10: popB} if h == HPC - 1 else \
                {1: fin, 3: popB, 7: popB, 11: popB}
            if pend_ag is not None:
                mids[2] = pend_ag
            fin = attn_unit(h, 1, mid=mids)
            if h % 2 == 1 and h < 5:
                pend_ag = (lambda p=h // 2: allgather(1, p))
            elif h == 5:
                pend_ag = (lambda: allgather(1, 2))
            elif h == 6:
                pend_ag = (lambda: allgather_single(6))
            else:
                pend_ag = None
            if h < HPC - 1:
                popB()
        fin()          # unit (7,1) normalize: the only non-deferred fin
        allgather_single(7)
        while fillQA or fillQB:
            popB()

        # ---- token-half 1 FFN in phases: ci{0,1,4,5} (pairs 0,1 landed long
        # ago), ci{2,6} (pair 2), the head-6 halves of ci{3,7} (single AG 6),
        # and only the head-7 halves (16 K=64 matmuls) wait on the last AG.
        with nc.named_scope("ffn1"):
            ftiles = []
            for bi in range(2):
                buf = sc_pool.tile([P, 2 * COH], f32, tag="sc", name=f"fpsc{bi}")
                ftiles += [buf[:, 0:COH], buf[:, COH:2 * COH]]
            buf = av_pool.tile([P, 2 * COH], f32, tag="av", name="fpav")
            ftiles += [buf[:, 0:COH], buf[:, COH:2 * COH]]
            ftiles += [flex_pool.tile([P, COH], f32, tag="flex", name=f"fpfx{i}")
                       for i in range(2)]
            for phase in ((0, 4, 1, 5), (2, 6)):
                for tl in range(8):
                    for ci in phase:
                        nc.tensor.matmul(
                            ftiles[tl], lhsT=ccout_sb[:, 1, ci, P * tl:P * (tl + 1)],
                            rhs=wfT_sb[:, ci, :],
                            start=(ci == 0), stop=False)
            for rows in (slice(0, 64), slice(64, 128)):   # head 6, then head 7
                for tl in range(8):
                    for ci in (3, 7):
                        nc.tensor.matmul(
                            ftiles[tl],
                            lhsT=ccout_sb[rows, 1, ci, P * tl:P * (tl + 1)],
                            rhs=wfT_sb[rows, ci, :],
                            start=False,
                            stop=(rows.start == 64 and ci == 7))
                    if rows.start == 64:
                        ysb = y_pool.tile([P, COH], b16, tag="y", name=f"y1_{tl}")
                        nc.vector.tensor_add(out=ysb, in0=ftiles[tl],
                                             in1=biasb_sb)
                        nc.vector.tensor_scalar_max(ysb, ysb, 0.0)
                        nc.sync.dma_start(
                            out=y.rearrange("(tt p) co -> tt p co", p=P)[8 + tl],
                            in_=ysb)

    nc.compile()
    return nc


def make_in_maps(x, Wq, Wk, Wv, Wf, bf):
    x = np.asarray(x, np.float32)
    mey_m = np.ascontiguousarray(-10000.0 * np.eye(P, dtype=np.float32)).astype(bf16)
    mls_m = np.ascontiguousarray(
        np.tril(np.ones((P, P), np.float32), -1)).astype(bf16)
    bf_f = np.asarray(bf, np.float32)
    wfT_f = np.asarray(Wf, np.float32).T
    in_maps = []
    for core in range(8):
        b, p = divmod(core, 2)
        sl = slice(HPC * p, HPC * (p + 1))
        in_maps.append({
            "xT": np.ascontiguousarray(x[b].T).astype(bf16),
            "wq": np.ascontiguousarray(
                np.asarray(Wq, np.float32)[:, sl].reshape(C, HPC * D)).astype(bf16),
            "wk": np.ascontiguousarray(
                np.asarray(Wk, np.float32)[:, sl].reshape(C, HPC * D)).astype(bf16),
            "wv": np.ascontiguousarray(
                np.asarray(Wv, np.float32)[:, sl].reshape(C, HPC * D)).astype(bf16),
            "wfT": np.ascontiguousarray(
                wfT_f[:, COH * p:COH * (p + 1)]).astype(bf16),
            "mey": mey_m,
            "mls": mls_m,
            "biasb": np.ascontiguousarray(np.tile(
                bf_f[None, COH * p:COH * (p + 1)], (P, 1))),
        })
    return in_maps


def run(x, Wq, Wk, Wv, Wf, bf, trace=False, **spmd_kwargs):
    from concourse.bass_utils import run_bass_kernel_spmd

    if "nc" not in _CACHE:
        _CACHE["nc"] = build_nc()
    nc = _CACHE["nc"]
    in_maps = make_in_maps(x, Wq, Wk, Wv, Wf, bf)
    res = run_bass_kernel_spmd(
        nc, in_maps, core_ids=list(range(8)), trace=trace, **spmd_kwargs)
    out = np.zeros((B, T, C), np.float32)
    for core in range(8):
        b, p = divmod(core, 2)
        out[b, :, COH * p:COH * (p + 1)] = \
            np.asarray(res.results[core]["y"]).astype(np.float32)
    return out, res


def kernel(x, Wq, Wk, Wv, Wf, bf):
    out, _ = run(x, Wq, Wk, Wv, Wf, bf, trace=False)
    return out


# revision 26
# speedup vs baseline: 1.2150x; 1.2150x over previous
"""Trainium2 Bass kernel for a dense transformer block (attention + ReLU FFN).

Reference computation (B=4, T=2048, C=1024, H=16, D=64):
    q,k,v = per-head projections of x;  causal softmax(q k^T / sqrt(C)) v;
    concat heads;  y = relu(out @ Wf.T + bf)

Sharding over 8 NeuronCores: core (2b+p) handles batch b with heads
[8p, 8p+8).  Attention runs causally over the full T on each core.  Pair
AllGathers (cores 2b/2b+1) share the attention outputs, and each core
runs the FFN for all 2048 tokens over its own half of the output
channels (the channel split is carried entirely by per-core input data -
every core executes an identical NEFF).

Layouts: scores are computed transposed ([s, t], keys on partitions) so
the exp() output feeds the AV matmul directly; V carries an appended
ones-column so row 64 of the AV accumulator is the softmax denominator;
causal masking is a -1e4 rank-128 matmul accumulated into the diagonal
score tile before exp. Compute dtype bf16 with fp32 PSUM accumulation.

Scheduling: a 48-matmul warmup burst on the (tiny, loaded-first) mask
constants heats the PE HAM clock-gate while the input DMAs stream;
projection/FFN matmul chunks are drained from filler queues inside the
attention j-loops so the PE never idles long enough to re-throttle; the
AllGathers run pair-wise and are emitted as soon as their two heads are
staged (the z-broadcast DMAs ride the Sync queue so the GpSimd queue
holds only collective triggers); the second-half FFN accumulates in
three phases so only the ci{3,7} matmuls depend on the last AllGather.
"""

import os
import sys

from collections import deque

import numpy as np
import ml_dtypes

for _p in ("/opt/trn_rl_repo", "/root/.axon_site/_ro/trn_rl_repo"):
    if os.path.isdir(_p) and _p not in sys.path:
        sys.path.append(_p)

B, T, C, H, D = 4, 2048, 1024, 16, 64
P = 128           # partitions
NCT = C // P      # 8 c-tiles
NTT = T // P      # 16 s/t-tiles
HPC = H // 2      # 8 heads per core
THALF = T // 2    # tokens per AllGather half
COH = C // 2      # output channels per core in the FFN
SCALE = float(C) ** -0.5
WARM_N = 48       # PE warmup matmuls (heats the HAM clock gate)

bf16 = ml_dtypes.bfloat16

_CACHE = {}


def build_nc():
    import concourse.bass as bass
    import concourse.tile as tile
    from concourse import bacc, mybir

    f32 = mybir.dt.float32
    b16 = mybir.dt.bfloat16
    EXP = mybir.ActivationFunctionType.Exp

    nc = bacc.Bacc("TRN2", target_bir_lowering=False, debug=False, num_devices=8)

    xT = nc.dram_tensor("xT", [C, T], b16, kind="ExternalInput").ap()
    wq = nc.dram_tensor("wq", [C, HPC * D], b16, kind="ExternalInput").ap()
    wk = nc.dram_tensor("wk", [C, HPC * D], b16, kind="ExternalInput").ap()
    wv = nc.dram_tensor("wv", [C, HPC * D], b16, kind="ExternalInput").ap()
    wfT = nc.dram_tensor("wfT", [C, COH], b16, kind="ExternalInput").ap()
    mey = nc.dram_tensor("mey", [P, P], b16, kind="ExternalInput").ap()
    mls_ = nc.dram_tensor("mls", [P, P], b16, kind="ExternalInput").ap()
    biasb = nc.dram_tensor("biasb", [P, COH], f32, kind="ExternalInput").ap()
    y = nc.dram_tensor("y", [T, COH], b16, kind="ExternalOutput").ap()
    # warmup sink: ExternalOutput so the warmup matmuls can't be DCE'd
    wsink = nc.dram_tensor("wsink", [P, P], f32, kind="ExternalOutput").ap()

    with tile.TileContext(nc) as tc, \
            tc.tile_pool(name="consts", bufs=1) as consts, \
            tc.tile_pool(name="dram", bufs=1, space="DRAM") as dram, \
            tc.tile_pool(name="sc_ps", bufs=2, space="PSUM") as sc_pool, \
            tc.tile_pool(name="av_ps", bufs=1, space="PSUM") as av_pool, \
            tc.tile_pool(name="flex_ps", bufs=2, space="PSUM") as flex_pool, \
            tc.tile_pool(name="wt", bufs=3) as wt_pool, \
            tc.tile_pool(name="norm", bufs=2) as norm_pool, \
            tc.tile_pool(name="yout", bufs=3) as y_pool:

        xT_sb = consts.tile([P, NCT, T], b16)
        wq_sb = consts.tile([P, NCT, HPC * D], b16)
        wk_sb = consts.tile([P, NCT, HPC * D], b16)
        wv_sb = consts.tile([P, NCT, HPC * D], b16)
        wfT_sb = consts.tile([P, NCT, COH], b16)
        mey_sb = consts.tile([P, P], b16)
        mls_sb = consts.tile([P, P], b16)
        biasb_sb = consts.tile([P, COH], f32)
        qT_sb = consts.tile([P, HPC // 2, T], b16)
        kT_sb = consts.tile([P, HPC // 2, T], b16)
        v_sb = consts.tile([P, NTT, HPC, D + 1], b16)
        ccout_sb = consts.tile([P, 2, NCT, THALF], b16)

        cc_in = [dram.tile([HPC * D, THALF], b16, name=f"cc_in{i}") for i in (0, 1)]
        cc_out = [[dram.tile([C // 4, THALF], b16, name=f"cc_out{th}_{p}")
                   for p in range(4)] for th in (0, 1)]
        cc_out1s = [dram.tile([P, THALF], b16, name=f"cc_out1s_{h}")
                    for h in (6, 7)]

        # ---- constant loads: per-ct pieces (parallel DMA sub-queues),
        # weights on the Sync HWDGE group and x on the Scalar HWDGE group so
        # the preamble isn't serialized on a single engine's DMA issue.
        nc.sync.dma_start(out=mey_sb, in_=mey)
        nc.sync.dma_start(out=mls_sb, in_=mls_)
        xT_r = xT.rearrange("(ct p) t -> ct p t", p=P)
        wq_r = wq.rearrange("(ct p) m -> ct p m", p=P)
        wk_r = wk.rearrange("(ct p) m -> ct p m", p=P)
        wv_r = wv.rearrange("(ct p) m -> ct p m", p=P)
        for ct in range(NCT):
            nc.sync.dma_start(out=wq_sb[:, ct, :], in_=wq_r[ct])
            nc.scalar.dma_start(out=xT_sb[:, ct, 0:512], in_=xT_r[ct][:, 0:512])
        for ct in range(NCT):
            nc.sync.dma_start(out=wk_sb[:, ct, :], in_=wk_r[ct])
            nc.scalar.dma_start(out=xT_sb[:, ct, 512:THALF],
                                in_=xT_r[ct][:, 512:THALF])
        for ct in range(NCT):
            nc.sync.dma_start(out=wv_sb[:, ct, :], in_=wv_r[ct])
        for ct in range(NCT):
            nc.scalar.dma_start(out=xT_sb[:, ct, THALF:THALF + 512],
                                in_=xT_r[ct][:, THALF:THALF + 512])
            nc.scalar.dma_start(out=xT_sb[:, ct, THALF + 512:T],
                                in_=xT_r[ct][:, THALF + 512:T])
        wfT_r = wfT.rearrange("(ct p) co -> ct p co", p=P)
        for ct in range(NCT):
            nc.sync.dma_start(out=wfT_sb[:, ct, :], in_=wfT_r[ct])
        nc.sync.dma_start(out=biasb_sb, in_=biasb)
        # ones in column 0 of v: row 0 of the AV accumulator is then the
        # softmax denominator, already at PSUM partition 0 where the
        # custom-DVE reciprocal needs it (no ScalarE extraction copy).
        nc.vector.memset(v_sb[:, :, :, 0:1], 1.0)

        # ---- PE warmup: dense matmul burst on the mask constants while the
        # big input DMAs stream; keeps the HAM gate at 8/8 for the real work.
        with nc.named_scope("warmup"):
            wps = flex_pool.tile([P, P], f32, tag="flex", name="warmps")
            for i in range(WARM_N):
                nc.tensor.matmul(wps, lhsT=mey_sb, rhs=mls_sb,
                                 start=(i == 0), stop=(i == WARM_N - 1))
            wsb = y_pool.tile([P, P], f32, tag="y", name="warmsb")
            nc.vector.tensor_copy(out=wsb, in_=wps)
            nc.sync.dma_start(out=wsink, in_=wsb)

        # ---- emission helpers ----------------------------------------------
        def v_proj(st):
          with nc.named_scope("vproj"):
            ps = flex_pool.tile([P, 512], f32, tag="flex", name=f"vps{st}")
            for ct in range(NCT):
                nc.tensor.matmul(
                    ps, lhsT=xT_sb[:, ct, P * st:P * (st + 1)],
                    rhs=wv_sb[:, ct, :],
                    start=(ct == 0), stop=(ct == NCT - 1))
            nc.vector.tensor_copy(out=v_sb[:, st, :, 1:D + 1],
                                  in_=ps.rearrange("p (h d) -> p h d", d=D))

        def qk_chunk(hp, i):
          with nc.named_scope("qkproj"):
            dst, w_t = ((qT_sb, wq_sb), (kT_sb, wk_sb))[i // 4]
            g = i % 4
            ps = flex_pool.tile([P, 512], f32, tag="flex", name=f"qkps{hp}_{i}")
            for ct in range(NCT):
                nc.tensor.matmul(
                    ps, lhsT=w_t[:, ct, hp * P:(hp + 1) * P],
                    rhs=xT_sb[:, ct, 512 * g:512 * (g + 1)],
                    start=(ct == 0), stop=(ct == NCT - 1))
            nc.vector.tensor_copy(out=dst[:, hp, 512 * g:512 * (g + 1)], in_=ps)

        def attn_unit(h, th, mid=None):
          with nc.named_scope(f"attn{th}_{h}"):
            hp, qh = divmod(h, 2)
            base = 64 * qh
            t0 = THALF * th
            av = av_pool.tile([P, THALF], f32, tag="av", name=f"av{h}_{th}")
            jmax = 8 * th + 8
            last_j = {0: 8 * th + 3, 1: jmax - 1}
            pend = None  # (j, pieces, wt) awaiting its AV emission

            def emit_av(ent):
                j, pieces, wt = ent
                for (o, e) in pieces:
                    region = 0 if o < 512 else 1
                    nc.tensor.matmul(
                        av[0:D + 1, o:e], lhsT=v_sb[:, j, h, :], rhs=wt[:, o:e],
                        start=(j == 0), stop=(j == last_j[region]))

            for j in range(jmax):
                off = max(0, P * j - t0)
                diag = P * j >= t0
                pieces = [(off, 512), (512, 1024)] if off < 512 \
                    else [(off, 1024)]
                sc = sc_pool.tile([P, THALF], f32, tag="sc", name=f"sc{h}_{th}_{j}")
                for pi, (o, e) in enumerate(pieces):
                    nc.tensor.matmul(
                        sc[:, o:e],
                        lhsT=kT_sb[base:base + 64, hp, P * j:P * (j + 1)],
                        rhs=qT_sb[base:base + 64, hp, t0 + o:t0 + e],
                        start=True, stop=not (diag and pi == 0))
                if diag:  # causal mask: accumulate -1e4 below the diagonal
                    nc.tensor.matmul(
                        sc[:, off:off + P], lhsT=mey_sb, rhs=mls_sb,
                        start=False, stop=True)
                wt = wt_pool.tile([P, THALF], b16, tag="wt", name=f"wt{h}_{th}_{j}")
                nc.scalar.activation(out=wt[:, off:THALF], in_=sc[:, off:THALF],
                                     func=EXP, scale=SCALE)
                if pend is not None:
                    emit_av(pend)
                if mid is not None and j in mid:
                    mid[j]()
                pend = (j, pieces, wt)
            emit_av(pend)
            # av row 0 is the softmax denominator (ones-column of v): the
            # custom-DVE reciprocal reads it straight from PSUM partition 0.
            zr = norm_pool.tile([1, THALF], f32, tag="zr", name=f"zrr{h}_{th}")
            nc.vector.reciprocal_approx_fast(out=zr, in_=av[0:1, 0:THALF])
            # evacuate the accumulator in one fast copy (frees the PSUM
            # slot for the next unit), then normalize u/Z off-path from SBUF
            avc = norm_pool.tile([D + 1, THALF], f32, tag="avc", name=f"avc{h}_{th}")
            nc.vector.tensor_copy(out=avc, in_=av[0:D + 1, 0:THALF])

            # broadcast 1/Z + stage on the GpSimd queue: it is otherwise idle
            # (collective triggers only), stalls there are harmless, and the
            # Sync/Scalar HWDGE groups stay free of late-produced DMAs whose
            # queue-counter semaphores would poison later consumers.
            zb = norm_pool.tile([D + 1, THALF], f32, tag="zb", name=f"zb{h}_{th}")
            zr_b = bass.AP(tensor=zr.tensor, offset=zr.offset,
                           ap=[list(zr.ap[0]), [0, D + 1], [1, THALF]])
            nc.gpsimd.dma_start(out=zb, in_=zr_b)
            # multiply rows 0:65 (partition-0-aligned for the DVE); row 0 is
            # Z * 1/Z and is simply not staged
            stage = norm_pool.tile([D + 1, THALF], b16, tag="stage", name=f"st{h}_{th}")
            nc.vector.tensor_mul(out=stage, in0=avc, in1=zb)
            nc.gpsimd.dma_start(out=cc_in[th][64 * h:64 * (h + 1), :],
                                in_=stage[1:D + 1, :])

        RG = [[0, 1], [2, 3], [4, 5], [6, 7]]

        def allgather(th, p):
          # head pair {2p, 2p+1} of token-half th -> ci-tiles p (rank0) and
          # 4+p (rank1), each complete
          with nc.named_scope(f"ag{th}_{p}"):
            import concourse.mybir as mybir_mod
            nc.gpsimd.collective_compute(
                "AllGather", mybir_mod.AluOpType.bypass, replica_groups=RG,
                ins=[cc_in[th][128 * p:128 * (p + 1), :].opt()],
                outs=[cc_out[th][p].opt()])
            cc_r = cc_out[th][p].rearrange("(ci p2) t -> ci p2 t", p2=P)
            nc.sync.dma_start(out=ccout_sb[:, th, p, :], in_=cc_r[0])
            nc.sync.dma_start(out=ccout_sb[:, th, 4 + p, :], in_=cc_r[1])

        def allgather_single(h):
          # single head h of token-half 1 -> 64-row halves of ci-tiles h//2
          # (rank0) and 4 + h//2 (rank1); keeps the tail AllGather small
          with nc.named_scope(f"ag1s_{h}"):
            import concourse.mybir as mybir_mod
            nc.gpsimd.collective_compute(
                "AllGather", mybir_mod.AluOpType.bypass, replica_groups=RG,
                ins=[cc_in[1][64 * h:64 * (h + 1), :].opt()],
                outs=[cc_out1s[h - 6].opt()])
            r0 = 64 * (h % 2)
            nc.sync.dma_start(out=ccout_sb[r0:r0 + 64, 1, h // 2, :],
                              in_=cc_out1s[h - 6][0:64, :])
            nc.sync.dma_start(out=ccout_sb[r0:r0 + 64, 1, 4 + h // 2, :],
                              in_=cc_out1s[h - 6][64:128, :])

        def ffn_tile0(tt):
          # full single-pass FFN tile for token-half 0 (all AGs landed)
          with nc.named_scope("ffn"):
            ps = flex_pool.tile([P, COH], f32, tag="flex", name=f"fps{tt}")
            for k, ci in enumerate((0, 4, 1, 5, 2, 6, 3, 7)):
                nc.tensor.matmul(
                    ps, lhsT=ccout_sb[:, 0, ci, P * tt:P * (tt + 1)],
                    rhs=wfT_sb[:, ci, :],
                    start=(k == 0), stop=(k == NCT - 1))
            ysb = y_pool.tile([P, COH], b16, tag="y", name=f"y{tt}")
            nc.vector.tensor_add(out=ysb, in0=ps, in1=biasb_sb)
            nc.vector.tensor_scalar_max(ysb, ysb, 0.0)
            nc.sync.dma_start(out=y.rearrange("(tt p) co -> tt p co", p=P)[tt],
                              in_=ysb)

        # ---- filler queues: projection/FFN chunks drained into the
        # attention j-loops to keep TensorE dense (and the HAM gate warm).
        # QA runs during token-half 0 (everything th1 units need up front);
        # QB runs inside the th1 units, sized to their exp-paced slack.
        fillQA = deque()   # th0-phase fillers (input-DMA gated only)
        fillQB = deque()   # th1-phase fillers (deadline-checked per unit)
        for hp in (1, 2, 3):
            for i in (0, 4, 1, 5):
                fillQA.append(lambda hp=hp, i=i: qk_chunk(hp, i))
        for st in range(8, NTT):
            fillQA.append(lambda st=st: v_proj(st))
        for i in (2, 6, 3, 7):
            fillQA.append(lambda i=i: qk_chunk(0, i))
        for hp in (1, 2, 3):   # hp g23 needed before th1 unit 2*hp
            for i in (2, 6, 3, 7):
                fillQB.append(lambda hp=hp, i=i: qk_chunk(hp, i))
        for tt in range(8):    # th0 FFN tiles (gated on the th0 AllGathers)
            fillQB.append(lambda tt=tt: ffn_tile0(tt))

        def popA():
            if fillQA:
                fillQA.popleft()()

        def popB():
            if fillQA:
                fillQA.popleft()()
            elif fillQB:
                fillQB.popleft()()

        # ---- emission order --------------------------------------------------
        # upfront: q/k for head-pair 0 over tokens 0:1024, v tiles 0:4
        for i in (0, 4, 1, 5):
            qk_chunk(0, i)
        for st in range(4):
            v_proj(st)
        # token-half 0 attention; v st4-7 finish inside unit 0
        attn_unit(0, 0, mid={1: lambda: v_proj(4), 2: lambda: v_proj(5),
                             3: lambda: v_proj(6), 4: lambda: v_proj(7),
                             5: popA, 7: popA})
        popA()
        for h in range(1, HPC):
            attn_unit(h, 0, mid={1: popA, 3: popA, 5: popA})
            if h % 2 == 1:
                allgather(0, h // 2)
            popA()
        while fillQA:  # all projection work must land before token-half 1
            popA()

        # token-half 1 attention: qk g23 chunks and th0 FFN tiles are spread
        # across the units to match their exp-paced slack
        attn_unit(0, 1, mid={2: popB, 5: popB, 8: popB, 11: popB})
        for h in range(1, HPC):
            mids = {2: popB, 6: popB, 10: popB} if h == HPC - 1 else \
                {1: popB, 3: popB, 7: popB, 11: popB}
            attn_unit(h, 1, mid=mids)
            if h % 2 == 1 and h < 5:
                allgather(1, h // 2)
            if h == 5:
                allgather(1, 2)
            if h == 6:
                allgather_single(6)
            if h < HPC - 1:
                popB()
        allgather_single(7)
        while fillQA or fillQB:
            popB()

        # ---- token-half 1 FFN in phases: ci{0,1,4,5} (pairs 0,1 landed long
        # ago), ci{2,6} (pair 2), the head-6 halves of ci{3,7} (single AG 6),
        # and only the head-7 halves (16 K=64 matmuls) wait on the last AG.
        with nc.named_scope("ffn1"):
            ftiles = []
            for bi in range(2):
                buf = sc_pool.tile([P, 2 * COH], f32, tag="sc", name=f"fpsc{bi}")
                ftiles += [buf[:, 0:COH], buf[:, COH:2 * COH]]
            buf = av_pool.tile([P, 2 * COH], f32, tag="av", name="fpav")
            ftiles += [buf[:, 0:COH], buf[:, COH:2 * COH]]
            ftiles += [flex_pool.tile([P, COH], f32, tag="flex", name=f"fpfx{i}")
                       for i in range(2)]
            for phase in ((0, 4, 1, 5), (2, 6)):
                for tl in range(8):
                    for ci in phase:
                        nc.tensor.matmul(
                            ftiles[tl], lhsT=ccout_sb[:, 1, ci, P * tl:P * (tl + 1)],
                            rhs=wfT_sb[:, ci, :],
                            start=(ci == 0), stop=False)
            for rows in (slice(0, 64), slice(64, 128)):   # head 6, then head 7
                for tl in range(8):
                    for ci in (3, 7):
                        nc.tensor.matmul(
                            ftiles[tl],
                            lhsT=ccout_sb[rows, 1, ci, P * tl:P * (tl + 1)],
                            rhs=wfT_sb[rows, ci, :],
                            start=False,
                            stop=(rows.start == 64 and ci == 7))
                    if rows.start == 64:
                        ysb = y_pool.tile([P, COH], b16, tag="y", name=f"y1_{tl}")
                        nc.vector.tensor_add(out=ysb, in0=ftiles[tl],
                                             in1=biasb_sb)
                        nc.vector.tensor_scalar_max(ysb, ysb, 0.0)
                        nc.sync.dma_start(
                            out=y.rearrange("(tt p) co -> tt p co", p=P)[8 + tl],
                            in_=ysb)

    nc.compile()
    return nc


def make_in_maps(x, Wq, Wk, Wv, Wf, bf):
    x = np.asarray(x, np.float32)
    mey_m = np.ascontiguousarray(-10000.0 * np.eye(P, dtype=np.float32)).astype(bf16)
    mls_m = np.ascontiguousarray(
        np.tril(np.ones((P, P), np.float32), -1)).astype(bf16)
    bf_f = np.asarray(bf, np.float32)
    wfT_f = np.asarray(Wf, np.float32).T
    in_maps = []
    for core in range(8):
        b, p = divmod(core, 2)
        sl = slice(HPC * p, HPC * (p + 1))
        in_maps.append({
            "xT": np.ascontiguousarray(x[b].T).astype(bf16),
            "wq": np.ascontiguousarray(
                np.asarray(Wq, np.float32)[:, sl].reshape(C, HPC * D)).astype(bf16),
            "wk": np.ascontiguousarray(
                np.asarray(Wk, np.float32)[:, sl].reshape(C, HPC * D)).astype(bf16),
            "wv": np.ascontiguousarray(
                np.asarray(Wv, np.float32)[:, sl].reshape(C, HPC * D)).astype(bf16),
            "wfT": np.ascontiguousarray(
                wfT_f[:, COH * p:COH * (p + 1)]).astype(bf16),
            "mey": mey_m,
            "mls": mls_m,
            "biasb": np.ascontiguousarray(np.tile(
                bf_f[None, COH * p:COH * (p + 1)], (P, 1))),
        })
    return in_maps


def run(x, Wq, Wk, Wv, Wf, bf, trace=False, **spmd_kwargs):
    from concourse.bass_utils import run_bass_kernel_spmd

    if "nc" not in _CACHE:
        _CACHE["nc"] = build_nc()
    nc = _CACHE["nc"]
    in_maps = make_in_maps(x, Wq, Wk, Wv, Wf, bf)
    res = run_bass_kernel_spmd(
        nc, in_maps, core_ids=list(range(8)), trace=trace, **spmd_kwargs)
    out = np.zeros((B, T, C), np.float32)
    for core in range(8):
        b, p = divmod(core, 2)
        out[b, :, COH * p:COH * (p + 1)] = \
            np.asarray(res.results[core]["y"]).astype(np.float32)
    return out, res


def kernel(x, Wq, Wk, Wv, Wf, bf):
    out, _ = run(x, Wq, Wk, Wv, Wf, bf, trace=False)
    return out


# revision 27
# speedup vs baseline: 1.2262x; 1.0092x over previous
"""Trainium2 Bass kernel for a dense transformer block (attention + ReLU FFN).

Reference computation (B=4, T=2048, C=1024, H=16, D=64):
    q,k,v = per-head projections of x;  causal softmax(q k^T / sqrt(C)) v;
    concat heads;  y = relu(out @ Wf.T + bf)

Sharding over 8 NeuronCores: core (2b+p) handles batch b with heads
[8p, 8p+8).  Attention runs causally over the full T on each core.  Pair
AllGathers (cores 2b/2b+1) share the attention outputs, and each core
runs the FFN for all 2048 tokens over its own half of the output
channels (the channel split is carried entirely by per-core input data -
every core executes an identical NEFF).

Layouts: scores are computed transposed ([s, t], keys on partitions) so
the exp() output feeds the AV matmul directly; V carries an appended
ones-column so row 64 of the AV accumulator is the softmax denominator;
causal masking is a -1e4 rank-128 matmul accumulated into the diagonal
score tile before exp. Compute dtype bf16 with fp32 PSUM accumulation.

Scheduling: a 48-matmul warmup burst on the (tiny, loaded-first) mask
constants heats the PE HAM clock-gate while the input DMAs stream;
projection/FFN matmul chunks are drained from filler queues inside the
attention j-loops so the PE never idles long enough to re-throttle; the
AllGathers run pair-wise and are emitted as soon as their two heads are
staged (the z-broadcast DMAs ride the Sync queue so the GpSimd queue
holds only collective triggers); the second-half FFN accumulates in
three phases so only the ci{3,7} matmuls depend on the last AllGather.
"""

import os
import sys

from collections import deque

import numpy as np
import ml_dtypes

for _p in ("/opt/trn_rl_repo", "/root/.axon_site/_ro/trn_rl_repo"):
    if os.path.isdir(_p) and _p not in sys.path:
        sys.path.append(_p)

B, T, C, H, D = 4, 2048, 1024, 16, 64
P = 128           # partitions
NCT = C // P      # 8 c-tiles
NTT = T // P      # 16 s/t-tiles
HPC = H // 2      # 8 heads per core
THALF = T // 2    # tokens per AllGather half
COH = C // 2      # output channels per core in the FFN
SCALE = float(C) ** -0.5
WARM_N = 48       # PE warmup matmuls (heats the HAM clock gate)

bf16 = ml_dtypes.bfloat16

_CACHE = {}


def build_nc():
    import concourse.bass as bass
    import concourse.tile as tile
    from concourse import bacc, mybir

    f32 = mybir.dt.float32
    b16 = mybir.dt.bfloat16
    EXP = mybir.ActivationFunctionType.Exp

    nc = bacc.Bacc("TRN2", target_bir_lowering=False, debug=False, num_devices=8)

    xT = nc.dram_tensor("xT", [C, T], b16, kind="ExternalInput").ap()
    wq = nc.dram_tensor("wq", [C, HPC * D], b16, kind="ExternalInput").ap()
    wk = nc.dram_tensor("wk", [C, HPC * D], b16, kind="ExternalInput").ap()
    wv = nc.dram_tensor("wv", [C, HPC * D], b16, kind="ExternalInput").ap()
    wfT = nc.dram_tensor("wfT", [C, COH], b16, kind="ExternalInput").ap()
    mey = nc.dram_tensor("mey", [P, P], b16, kind="ExternalInput").ap()
    mls_ = nc.dram_tensor("mls", [P, P], b16, kind="ExternalInput").ap()
    biasb = nc.dram_tensor("biasb", [P, COH], f32, kind="ExternalInput").ap()
    y = nc.dram_tensor("y", [T, COH], b16, kind="ExternalOutput").ap()
    # warmup sink: ExternalOutput so the warmup matmuls can't be DCE'd
    wsink = nc.dram_tensor("wsink", [P, P], f32, kind="ExternalOutput").ap()

    with tile.TileContext(nc) as tc, \
            tc.tile_pool(name="consts", bufs=1) as consts, \
            tc.tile_pool(name="dram", bufs=1, space="DRAM") as dram, \
            tc.tile_pool(name="sc_ps", bufs=2, space="PSUM") as sc_pool, \
            tc.tile_pool(name="av_ps", bufs=1, space="PSUM") as av_pool, \
            tc.tile_pool(name="flex_ps", bufs=2, space="PSUM") as flex_pool, \
            tc.tile_pool(name="wt", bufs=3) as wt_pool, \
            tc.tile_pool(name="norm", bufs=2) as norm_pool, \
            tc.tile_pool(name="yout", bufs=3) as y_pool:

        xT_sb = consts.tile([P, NCT, T], b16)
        wq_sb = consts.tile([P, NCT, HPC * D], b16)
        wk_sb = consts.tile([P, NCT, HPC * D], b16)
        wv_sb = consts.tile([P, NCT, HPC * D], b16)
        wfT_sb = consts.tile([P, NCT, COH], b16)
        mey_sb = consts.tile([P, P], b16)
        mls_sb = consts.tile([P, P], b16)
        biasb_sb = consts.tile([P, COH], f32)
        qT_sb = consts.tile([P, HPC // 2, T], b16)
        kT_sb = consts.tile([P, HPC // 2, T], b16)
        v_sb = consts.tile([P, NTT, HPC, D + 1], b16)
        ccout_sb = consts.tile([P, 2, NCT, THALF], b16)

        cc_in = [dram.tile([HPC * D, THALF], b16, name=f"cc_in{i}") for i in (0, 1)]
        cc_out = [[dram.tile([C // 4, THALF], b16, name=f"cc_out{th}_{p}")
                   for p in range(4)] for th in (0, 1)]
        cc_out1s = [dram.tile([P, THALF], b16, name=f"cc_out1s_{h}")
                    for h in (6, 7)]

        # ---- constant loads: per-ct pieces, ALL on the Sync HWDGE group (the
        # Scalar group executes DMAs an order of magnitude slower in
        # aggregate); ordered first-needed-first so issue serialization only
        # delays late consumers.
        nc.sync.dma_start(out=mey_sb, in_=mey)
        nc.sync.dma_start(out=mls_sb, in_=mls_)
        xT_r = xT.rearrange("(ct p) t -> ct p t", p=P)
        wq_r = wq.rearrange("(ct p) m -> ct p m", p=P)
        wk_r = wk.rearrange("(ct p) m -> ct p m", p=P)
        wv_r = wv.rearrange("(ct p) m -> ct p m", p=P)
        for ct in range(NCT):
            nc.sync.dma_start(out=wq_sb[:, ct, :], in_=wq_r[ct])
            nc.sync.dma_start(out=xT_sb[:, ct, 0:512], in_=xT_r[ct][:, 0:512])
        for ct in range(NCT):
            nc.sync.dma_start(out=wk_sb[:, ct, :], in_=wk_r[ct])
            nc.sync.dma_start(out=xT_sb[:, ct, 512:THALF],
                              in_=xT_r[ct][:, 512:THALF])
        for ct in range(NCT):
            nc.sync.dma_start(out=wv_sb[:, ct, :], in_=wv_r[ct])
        for ct in range(NCT):
            nc.sync.dma_start(out=xT_sb[:, ct, THALF:THALF + 512],
                              in_=xT_r[ct][:, THALF:THALF + 512])
            nc.sync.dma_start(out=xT_sb[:, ct, THALF + 512:T],
                              in_=xT_r[ct][:, THALF + 512:T])
        wfT_r = wfT.rearrange("(ct p) co -> ct p co", p=P)
        for ct in range(NCT):
            nc.sync.dma_start(out=wfT_sb[:, ct, :], in_=wfT_r[ct])
        nc.sync.dma_start(out=biasb_sb, in_=biasb)
        # ones in column 0 of v: row 0 of the AV accumulator is then the
        # softmax denominator, already at PSUM partition 0 where the
        # custom-DVE reciprocal needs it (no ScalarE extraction copy).
        nc.vector.memset(v_sb[:, :, :, 0:1], 1.0)

        # ---- PE warmup: dense matmul burst on the mask constants while the
        # big input DMAs stream; keeps the HAM gate at 8/8 for the real work.
        with nc.named_scope("warmup"):
            wps = flex_pool.tile([P, P], f32, tag="flex", name="warmps")
            for i in range(WARM_N):
                nc.tensor.matmul(wps, lhsT=mey_sb, rhs=mls_sb,
                                 start=(i == 0), stop=(i == WARM_N - 1))
            wsb = y_pool.tile([P, P], f32, tag="y", name="warmsb")
            nc.vector.tensor_copy(out=wsb, in_=wps)
            nc.sync.dma_start(out=wsink, in_=wsb)

        # ---- emission helpers ----------------------------------------------
        def v_proj(st):
          with nc.named_scope("vproj"):
            ps = flex_pool.tile([P, 512], f32, tag="flex", name=f"vps{st}")
            for ct in range(NCT):
                nc.tensor.matmul(
                    ps, lhsT=xT_sb[:, ct, P * st:P * (st + 1)],
                    rhs=wv_sb[:, ct, :],
                    start=(ct == 0), stop=(ct == NCT - 1))
            nc.vector.tensor_copy(out=v_sb[:, st, :, 1:D + 1],
                                  in_=ps.rearrange("p (h d) -> p h d", d=D))

        def qk_chunk(hp, i):
          with nc.named_scope("qkproj"):
            dst, w_t = ((qT_sb, wq_sb), (kT_sb, wk_sb))[i // 4]
            g = i % 4
            ps = flex_pool.tile([P, 512], f32, tag="flex", name=f"qkps{hp}_{i}")
            for ct in range(NCT):
                nc.tensor.matmul(
                    ps, lhsT=w_t[:, ct, hp * P:(hp + 1) * P],
                    rhs=xT_sb[:, ct, 512 * g:512 * (g + 1)],
                    start=(ct == 0), stop=(ct == NCT - 1))
            nc.vector.tensor_copy(out=dst[:, hp, 512 * g:512 * (g + 1)], in_=ps)

        def attn_unit(h, th, mid=None):
          with nc.named_scope(f"attn{th}_{h}"):
            hp, qh = divmod(h, 2)
            base = 64 * qh
            t0 = THALF * th
            av = av_pool.tile([P, THALF], f32, tag="av", name=f"av{h}_{th}")
            jmax = 8 * th + 8
            last_j = {0: 8 * th + 3, 1: jmax - 1}
            pend = None  # (j, pieces, wt) awaiting its AV emission

            def emit_av(ent):
                j, pieces, wt = ent
                for (o, e) in pieces:
                    region = 0 if o < 512 else 1
                    nc.tensor.matmul(
                        av[0:D + 1, o:e], lhsT=v_sb[:, j, h, :], rhs=wt[:, o:e],
                        start=(j == 0), stop=(j == last_j[region]))

            for j in range(jmax):
                off = max(0, P * j - t0)
                diag = P * j >= t0
                pieces = [(off, 512), (512, 1024)] if off < 512 \
                    else [(off, 1024)]
                sc = sc_pool.tile([P, THALF], f32, tag="sc", name=f"sc{h}_{th}_{j}")
                for pi, (o, e) in enumerate(pieces):
                    nc.tensor.matmul(
                        sc[:, o:e],
                        lhsT=kT_sb[base:base + 64, hp, P * j:P * (j + 1)],
                        rhs=qT_sb[base:base + 64, hp, t0 + o:t0 + e],
                        start=True, stop=not (diag and pi == 0))
                if diag:  # causal mask: accumulate -1e4 below the diagonal
                    nc.tensor.matmul(
                        sc[:, off:off + P], lhsT=mey_sb, rhs=mls_sb,
                        start=False, stop=True)
                wt = wt_pool.tile([P, THALF], b16, tag="wt", name=f"wt{h}_{th}_{j}")
                nc.scalar.activation(out=wt[:, off:THALF], in_=sc[:, off:THALF],
                                     func=EXP, scale=SCALE)
                if pend is not None:
                    emit_av(pend)
                if mid is not None and j in mid:
                    mid[j]()
                pend = (j, pieces, wt)
            emit_av(pend)
            # av row 0 is the softmax denominator (ones-column of v): the
            # custom-DVE reciprocal reads it straight from PSUM partition 0.
            zr = norm_pool.tile([1, THALF], f32, tag="zr", name=f"zrr{h}_{th}")
            nc.vector.reciprocal_approx_fast(out=zr, in_=av[0:1, 0:THALF])
            # evacuate the accumulator in one fast copy (frees the PSUM
            # slot for the next unit), then normalize u/Z off-path from SBUF
            avc = norm_pool.tile([D + 1, THALF], f32, tag="avc", name=f"avc{h}_{th}")
            nc.vector.tensor_copy(out=avc, in_=av[0:D + 1, 0:THALF])

            # broadcast 1/Z + stage on the GpSimd queue: it is otherwise idle
            # (collective triggers only), stalls there are harmless, and the
            # Sync/Scalar HWDGE groups stay free of late-produced DMAs whose
            # queue-counter semaphores would poison later consumers.
            zb = norm_pool.tile([D + 1, THALF], f32, tag="zb", name=f"zb{h}_{th}")
            zr_b = bass.AP(tensor=zr.tensor, offset=zr.offset,
                           ap=[list(zr.ap[0]), [0, D + 1], [1, THALF]])
            nc.gpsimd.dma_start(out=zb, in_=zr_b)
            # multiply rows 0:65 (partition-0-aligned for the DVE); row 0 is
            # Z * 1/Z and is simply not staged
            stage = norm_pool.tile([D + 1, THALF], b16, tag="stage", name=f"st{h}_{th}")
            nc.vector.tensor_mul(out=stage, in0=avc, in1=zb)
            nc.gpsimd.dma_start(out=cc_in[th][64 * h:64 * (h + 1), :],
                                in_=stage[1:D + 1, :])

        RG = [[0, 1], [2, 3], [4, 5], [6, 7]]

        def allgather(th, p):
          # head pair {2p, 2p+1} of token-half th -> ci-tiles p (rank0) and
          # 4+p (rank1), each complete
          with nc.named_scope(f"ag{th}_{p}"):
            import concourse.mybir as mybir_mod
            nc.gpsimd.collective_compute(
                "AllGather", mybir_mod.AluOpType.bypass, replica_groups=RG,
                ins=[cc_in[th][128 * p:128 * (p + 1), :].opt()],
                outs=[cc_out[th][p].opt()])
            cc_r = cc_out[th][p].rearrange("(ci p2) t -> ci p2 t", p2=P)
            nc.sync.dma_start(out=ccout_sb[:, th, p, :], in_=cc_r[0])
            nc.sync.dma_start(out=ccout_sb[:, th, 4 + p, :], in_=cc_r[1])

        def allgather_single(h):
          # single head h of token-half 1 -> 64-row halves of ci-tiles h//2
          # (rank0) and 4 + h//2 (rank1); keeps the tail AllGather small
          with nc.named_scope(f"ag1s_{h}"):
            import concourse.mybir as mybir_mod
            nc.gpsimd.collective_compute(
                "AllGather", mybir_mod.AluOpType.bypass, replica_groups=RG,
                ins=[cc_in[1][64 * h:64 * (h + 1), :].opt()],
                outs=[cc_out1s[h - 6].opt()])
            r0 = 64 * (h % 2)
            nc.sync.dma_start(out=ccout_sb[r0:r0 + 64, 1, h // 2, :],
                              in_=cc_out1s[h - 6][0:64, :])
            nc.sync.dma_start(out=ccout_sb[r0:r0 + 64, 1, 4 + h // 2, :],
                              in_=cc_out1s[h - 6][64:128, :])

        def ffn_tile0(tt):
          # full single-pass FFN tile for token-half 0 (all AGs landed)
          with nc.named_scope("ffn"):
            ps = flex_pool.tile([P, COH], f32, tag="flex", name=f"fps{tt}")
            for k, ci in enumerate((0, 4, 1, 5, 2, 6, 3, 7)):
                nc.tensor.matmul(
                    ps, lhsT=ccout_sb[:, 0, ci, P * tt:P * (tt + 1)],
                    rhs=wfT_sb[:, ci, :],
                    start=(k == 0), stop=(k == NCT - 1))
            ysb = y_pool.tile([P, COH], b16, tag="y", name=f"y{tt}")
            nc.vector.tensor_add(out=ysb, in0=ps, in1=biasb_sb)
            nc.vector.tensor_scalar_max(ysb, ysb, 0.0)
            nc.sync.dma_start(out=y.rearrange("(tt p) co -> tt p co", p=P)[tt],
                              in_=ysb)

        # ---- filler queues: projection/FFN chunks drained into the
        # attention j-loops to keep TensorE dense (and the HAM gate warm).
        # QA runs during token-half 0 (everything th1 units need up front);
        # QB runs inside the th1 units, sized to their exp-paced slack.
        fillQA = deque()   # th0-phase fillers (input-DMA gated only)
        fillQB = deque()   # th1-phase fillers (deadline-checked per unit)
        for hp in (1, 2, 3):
            for i in (0, 4, 1, 5):
                fillQA.append(lambda hp=hp, i=i: qk_chunk(hp, i))
        for st in range(8, NTT):
            fillQA.append(lambda st=st: v_proj(st))
        for i in (2, 6, 3, 7):
            fillQA.append(lambda i=i: qk_chunk(0, i))
        for hp in (1, 2, 3):   # hp g23 needed before th1 unit 2*hp
            for i in (2, 6, 3, 7):
                fillQB.append(lambda hp=hp, i=i: qk_chunk(hp, i))
        for tt in range(8):    # th0 FFN tiles (gated on the th0 AllGathers)
            fillQB.append(lambda tt=tt: ffn_tile0(tt))

        def popA():
            if fillQA:
                fillQA.popleft()()

        def popB():
            if fillQA:
                fillQA.popleft()()
            elif fillQB:
                fillQB.popleft()()

        # ---- emission order --------------------------------------------------
        # upfront: q/k for head-pair 0 over tokens 0:1024, v tiles 0:4
        for i in (0, 4, 1, 5):
            qk_chunk(0, i)
        for st in range(4):
            v_proj(st)
        # token-half 0 attention; v st4-7 finish inside unit 0
        attn_unit(0, 0, mid={1: lambda: v_proj(4), 2: lambda: v_proj(5),
                             3: lambda: v_proj(6), 4: lambda: v_proj(7),
                             5: popA, 7: popA})
        popA()
        for h in range(1, HPC):
            attn_unit(h, 0, mid={1: popA, 3: popA, 5: popA})
            if h % 2 == 1:
                allgather(0, h // 2)
            popA()
        while fillQA:  # all projection work must land before token-half 1
            popA()

        # token-half 1 attention: qk g23 chunks and th0 FFN tiles are spread
        # across the units to match their exp-paced slack
        attn_unit(0, 1, mid={2: popB, 5: popB, 8: popB, 11: popB})
        for h in range(1, HPC):
            mids = {2: popB, 6: popB, 10: popB} if h == HPC - 1 else \
                {1: popB, 3: popB, 7: popB, 11: popB}
            attn_unit(h, 1, mid=mids)
            if h % 2 == 1 and h < 5:
                allgather(1, h // 2)
            if h == 5:
                allgather(1, 2)
            if h == 6:
                allgather_single(6)
            if h < HPC - 1:
                popB()
        allgather_single(7)
        while fillQA or fillQB:
            popB()

        # ---- token-half 1 FFN in phases: ci{0,1,4,5} (pairs 0,1 landed long
        # ago), ci{2,6} (pair 2), the head-6 halves of ci{3,7} (single AG 6),
        # and only the head-7 halves (16 K=64 matmuls) wait on the last AG.
        with nc.named_scope("ffn1"):
            ftiles = []
            for bi in range(2):
                buf = sc_pool.tile([P, 2 * COH], f32, tag="sc", name=f"fpsc{bi}")
                ftiles += [buf[:, 0:COH], buf[:, COH:2 * COH]]
            buf = av_pool.tile([P, 2 * COH], f32, tag="av", name="fpav")
            ftiles += [buf[:, 0:COH], buf[:, COH:2 * COH]]
            ftiles += [flex_pool.tile([P, COH], f32, tag="flex", name=f"fpfx{i}")
                       for i in range(2)]
            for phase in ((0, 4, 1, 5), (2, 6)):
                for tl in range(8):
                    for ci in phase:
                        nc.tensor.matmul(
                            ftiles[tl], lhsT=ccout_sb[:, 1, ci, P * tl:P * (tl + 1)],
                            rhs=wfT_sb[:, ci, :],
                            start=(ci == 0), stop=False)
            for rows in (slice(0, 64), slice(64, 128)):   # head 6, then head 7
                for tl in range(8):
                    for ci in (3, 7):
                        nc.tensor.matmul(
                            ftiles[tl],
                            lhsT=ccout_sb[rows, 1, ci, P * tl:P * (tl + 1)],
                            rhs=wfT_sb[rows, ci, :],
                            start=False,
                            stop=(rows.start == 64 and ci == 7))
                    if rows.start == 64:
                        ysb = y_pool.tile([P, COH], b16, tag="y", name=f"y1_{tl}")
                        nc.vector.tensor_add(out=ysb, in0=ftiles[tl],
                                             in1=biasb_sb)
                        nc.vector.tensor_scalar_max(ysb, ysb, 0.0)
                        nc.sync.dma_start(
                            out=y.rearrange("(tt p) co -> tt p co", p=P)[8 + tl],
                            in_=ysb)

    nc.compile()
    return nc


def make_in_maps(x, Wq, Wk, Wv, Wf, bf):
    x = np.asarray(x, np.float32)
    mey_m = np.ascontiguousarray(-10000.0 * np.eye(P, dtype=np.float32)).astype(bf16)
    mls_m = np.ascontiguousarray(
        np.tril(np.ones((P, P), np.float32), -1)).astype(bf16)
    bf_f = np.asarray(bf, np.float32)
    wfT_f = np.asarray(Wf, np.float32).T
    in_maps = []
    for core in range(8):
        b, p = divmod(core, 2)
        sl = slice(HPC * p, HPC * (p + 1))
        in_maps.append({
            "xT": np.ascontiguousarray(x[b].T).astype(bf16),
            "wq": np.ascontiguousarray(
                np.asarray(Wq, np.float32)[:, sl].reshape(C, HPC * D)).astype(bf16),
            "wk": np.ascontiguousarray(
                np.asarray(Wk, np.float32)[:, sl].reshape(C, HPC * D)).astype(bf16),
            "wv": np.ascontiguousarray(
                np.asarray(Wv, np.float32)[:, sl].reshape(C, HPC * D)).astype(bf16),
            "wfT": np.ascontiguousarray(
                wfT_f[:, COH * p:COH * (p + 1)]).astype(bf16),
            "mey": mey_m,
            "mls": mls_m,
            "biasb": np.ascontiguousarray(np.tile(
                bf_f[None, COH * p:COH * (p + 1)], (P, 1))),
        })
    return in_maps


def run(x, Wq, Wk, Wv, Wf, bf, trace=False, **spmd_kwargs):
    from concourse.bass_utils import run_bass_kernel_spmd

    if "nc" not in _CACHE:
        _CACHE["nc"] = build_nc()
    nc = _CACHE["nc"]
    in_maps = make_in_maps(x, Wq, Wk, Wv, Wf, bf)
    res = run_bass_kernel_spmd(
        nc, in_maps, core_ids=list(range(8)), trace=trace, **spmd_kwargs)
    out = np.zeros((B, T, C), np.float32)
    for core in range(8):
        b, p = divmod(core, 2)
        out[b, :, COH * p:COH * (p + 1)] = \
            np.asarray(res.results[core]["y"]).astype(np.float32)
    return out, res


def kernel(x, Wq, Wk, Wv, Wf, bf):
    out, _ = run(x, Wq, Wk, Wv, Wf, bf, trace=False)
    return out


# revision 33
# speedup vs baseline: 1.3231x; 1.0790x over previous
"""Trainium2 Bass kernel for a dense transformer block (attention + ReLU FFN).

Reference computation (B=4, T=2048, C=1024, H=16, D=64):
    q,k,v = per-head projections of x;  causal softmax(q k^T / sqrt(C)) v;
    concat heads;  y = relu(out @ Wf.T + bf)

Sharding over 8 NeuronCores: core (2b+p) handles batch b with heads
[8p, 8p+8).  Attention runs causally over the full T on each core.  Pair
AllGathers (cores 2b/2b+1) share the attention outputs, and each core
runs the FFN for all 2048 tokens over its own half of the output
channels (the channel split is carried entirely by per-core input data -
every core executes an identical NEFF).

Layouts: scores are computed transposed ([s, t], keys on partitions) so
the exp() output feeds the AV matmul directly; V carries an appended
ones-column so row 64 of the AV accumulator is the softmax denominator;
causal masking is a -1e4 rank-128 matmul accumulated into the diagonal
score tile before exp. Compute dtype bf16 with fp32 PSUM accumulation.

Scheduling: a 48-matmul warmup burst on the (tiny, loaded-first) mask
constants heats the PE HAM clock-gate while the input DMAs stream;
projection/FFN matmul chunks are drained from filler queues inside the
attention j-loops so the PE never idles long enough to re-throttle; the
AllGathers run pair-wise and are emitted as soon as their two heads are
staged (the z-broadcast DMAs ride the Sync queue so the GpSimd queue
holds only collective triggers); the second-half FFN accumulates in
three phases so only the ci{3,7} matmuls depend on the last AllGather.
"""

import os
import sys

from collections import deque

import numpy as np
import ml_dtypes

for _p in ("/opt/trn_rl_repo", "/root/.axon_site/_ro/trn_rl_repo"):
    if os.path.isdir(_p) and _p not in sys.path:
        sys.path.append(_p)

B, T, C, H, D = 4, 2048, 1024, 16, 64
P = 128           # partitions
NCT = C // P      # 8 c-tiles
NTT = T // P      # 16 s/t-tiles
HPC = H // 2      # 8 heads per core
THALF = T // 2    # tokens per AllGather half
COH = C // 2      # output channels per core in the FFN
SCALE = float(C) ** -0.5
WARM_N = 160      # PE warmup matmuls (heats the HAM clock gate until data lands)

bf16 = ml_dtypes.bfloat16

_CACHE = {}


def build_nc():
    import concourse.bass as bass
    import concourse.tile as tile
    from concourse import bacc, mybir

    f32 = mybir.dt.float32
    b16 = mybir.dt.bfloat16
    EXP = mybir.ActivationFunctionType.Exp

    nc = bacc.Bacc("TRN2", target_bir_lowering=False, debug=False, num_devices=8)

    # inputs are host-packed to (partition, ct, col) so every DMA moves
    # 4-8KB contiguous lines per partition (1KB strided lines run ~10x
    # slower through the DMA engines)
    xg = [nc.dram_tensor(f"xg{g}", [P, NCT * 512], b16, kind="ExternalInput").ap()
          for g in range(4)]
    wq = nc.dram_tensor("wq", [P, NCT * HPC * D], b16, kind="ExternalInput").ap()
    wk = nc.dram_tensor("wk", [P, NCT * HPC * D], b16, kind="ExternalInput").ap()
    wv = nc.dram_tensor("wv", [P, NCT * HPC * D], b16, kind="ExternalInput").ap()
    wfT = nc.dram_tensor("wfT", [P, NCT * COH], b16, kind="ExternalInput").ap()
    mey = nc.dram_tensor("mey", [P, P], b16, kind="ExternalInput").ap()
    mls_ = nc.dram_tensor("mls", [P, P], b16, kind="ExternalInput").ap()
    biasb = nc.dram_tensor("biasb", [P, COH], f32, kind="ExternalInput").ap()
    y = nc.dram_tensor("y", [T, COH], b16, kind="ExternalOutput").ap()
    # warmup sink: ExternalOutput so the warmup matmuls can't be DCE'd
    wsink = nc.dram_tensor("wsink", [P, P], f32, kind="ExternalOutput").ap()

    with tile.TileContext(nc) as tc, \
            tc.tile_pool(name="consts", bufs=1) as consts, \
            tc.tile_pool(name="dram", bufs=1, space="DRAM") as dram, \
            tc.tile_pool(name="sc_ps", bufs=2, space="PSUM") as sc_pool, \
            tc.tile_pool(name="av_ps", bufs=1, space="PSUM") as av_pool, \
            tc.tile_pool(name="flex_ps", bufs=2, space="PSUM") as flex_pool, \
            tc.tile_pool(name="wt", bufs=3) as wt_pool, \
            tc.tile_pool(name="norm", bufs=2) as norm_pool, \
            tc.tile_pool(name="yout", bufs=3) as y_pool:

        xT_sb = consts.tile([P, NCT, T], b16)
        wq_sb = consts.tile([P, NCT, HPC * D], b16)
        wk_sb = consts.tile([P, NCT, HPC * D], b16)
        wv_sb = consts.tile([P, NCT, HPC * D], b16)
        wfT_sb = consts.tile([P, NCT, COH], b16)
        mey_sb = consts.tile([P, P], b16)
        mls_sb = consts.tile([P, P], b16)
        biasb_sb = consts.tile([P, COH], f32)
        qT_sb = consts.tile([P, HPC // 2, T], b16)
        kT_sb = consts.tile([P, HPC // 2, T], b16)
        v_sb = consts.tile([P, NTT, HPC, D + 1], b16)
        ccout_sb = consts.tile([P, 2, NCT, THALF], b16)

        cc_in = [dram.tile([HPC * D, THALF], b16, name=f"cc_in{i}") for i in (0, 1)]
        cc_out = [[dram.tile([C // 4, THALF], b16, name=f"cc_out{th}_{p}")
                   for p in range(4)] for th in (0, 1)]
        cc_out1s = [dram.tile([P, THALF], b16, name=f"cc_out1s_{h}")
                    for h in (6, 7)]

        # ---- constant loads on the Sync HWDGE group, first-needed-first,
        # halved for queue parallelism on the critical early tensors.
        nc.sync.dma_start(out=mey_sb, in_=mey)
        nc.sync.dma_start(out=mls_sb, in_=mls_)

        def load_packed(sb, dram, n_split, cols):
            # sb is [P, NCT, cols]-shaped view; dram is [P, NCT*cols] packed
            step = NCT // n_split
            for s in range(n_split):
                nc.sync.dma_start(
                    out=sb[:, s * step:(s + 1) * step, :],
                    in_=dram[:, s * step * cols:(s + 1) * step * cols])

        def load_xg(g, n_split):
            step = NCT // n_split
            for s in range(n_split):
                nc.sync.dma_start(
                    out=xT_sb[:, s * step:(s + 1) * step, 512 * g:512 * (g + 1)],
                    in_=xg[g][:, s * step * 512:(s + 1) * step * 512])

        load_packed(wq_sb, wq, 2, HPC * D)
        load_xg(0, 2)
        load_packed(wk_sb, wk, 2, HPC * D)
        load_xg(1, 2)
        load_packed(wv_sb, wv, 2, HPC * D)
        load_xg(2, 2)
        load_xg(3, 2)
        load_packed(wfT_sb, wfT, 2, COH)
        nc.sync.dma_start(out=biasb_sb, in_=biasb)
        # ones in column 0 of v: row 0 of the AV accumulator is then the
        # softmax denominator, already at PSUM partition 0 where the
        # custom-DVE reciprocal needs it (no ScalarE extraction copy).
        nc.vector.memset(v_sb[:, :, :, 0:1], 1.0)

        # ---- PE warmup: dense matmul burst on the mask constants while the
        # big input DMAs stream; keeps the HAM gate at 8/8 for the real work.
        with nc.named_scope("warmup"):
            wps = flex_pool.tile([P, P], f32, tag="flex", name="warmps")
            for i in range(WARM_N):
                nc.tensor.matmul(wps, lhsT=mey_sb, rhs=mls_sb,
                                 start=(i == 0), stop=(i == WARM_N - 1))
            wsb = y_pool.tile([P, P], f32, tag="y", name="warmsb")
            nc.vector.tensor_copy(out=wsb, in_=wps)
            nc.sync.dma_start(out=wsink, in_=wsb)

        # ---- emission helpers ----------------------------------------------
        def v_proj(st):
          with nc.named_scope("vproj"):
            ps = flex_pool.tile([P, 512], f32, tag="flex", name=f"vps{st}")
            for ct in range(NCT):
                nc.tensor.matmul(
                    ps, lhsT=xT_sb[:, ct, P * st:P * (st + 1)],
                    rhs=wv_sb[:, ct, :],
                    start=(ct == 0), stop=(ct == NCT - 1))
            nc.vector.tensor_copy(out=v_sb[:, st, :, 1:D + 1],
                                  in_=ps.rearrange("p (h d) -> p h d", d=D))

        def qk_chunk(hp, i):
          with nc.named_scope("qkproj"):
            dst, w_t = ((qT_sb, wq_sb), (kT_sb, wk_sb))[i // 4]
            g = i % 4
            ps = flex_pool.tile([P, 512], f32, tag="flex", name=f"qkps{hp}_{i}")
            for ct in range(NCT):
                nc.tensor.matmul(
                    ps, lhsT=w_t[:, ct, hp * P:(hp + 1) * P],
                    rhs=xT_sb[:, ct, 512 * g:512 * (g + 1)],
                    start=(ct == 0), stop=(ct == NCT - 1))
            nc.vector.tensor_copy(out=dst[:, hp, 512 * g:512 * (g + 1)], in_=ps)

        def attn_unit(h, th, mid=None):
          with nc.named_scope(f"attn{th}_{h}"):
            hp, qh = divmod(h, 2)
            base = 64 * qh
            t0 = THALF * th
            av = av_pool.tile([P, THALF], f32, tag="av", name=f"av{h}_{th}")
            jmax = 8 * th + 8
            last_j = {0: 8 * th + 3, 1: jmax - 1}
            pend = None  # (j, pieces, wt) awaiting its AV emission

            def emit_av(ent):
                j, pieces, wt = ent
                for (o, e) in pieces:
                    region = 0 if o < 512 else 1
                    nc.tensor.matmul(
                        av[0:D + 1, o:e], lhsT=v_sb[:, j, h, :], rhs=wt[:, o:e],
                        start=(j == 0), stop=(j == last_j[region]))

            for j in range(jmax):
                off = max(0, P * j - t0)
                diag = P * j >= t0
                pieces = [(off, 512), (512, 1024)] if off < 512 \
                    else [(off, 1024)]
                sc = sc_pool.tile([P, THALF], f32, tag="sc", name=f"sc{h}_{th}_{j}")
                for pi, (o, e) in enumerate(pieces):
                    nc.tensor.matmul(
                        sc[:, o:e],
                        lhsT=kT_sb[base:base + 64, hp, P * j:P * (j + 1)],
                        rhs=qT_sb[base:base + 64, hp, t0 + o:t0 + e],
                        start=True, stop=not (diag and pi == 0))
                if diag:  # causal mask: accumulate -1e4 below the diagonal
                    nc.tensor.matmul(
                        sc[:, off:off + P], lhsT=mey_sb, rhs=mls_sb,
                        start=False, stop=True)
                wt = wt_pool.tile([P, THALF], b16, tag="wt", name=f"wt{h}_{th}_{j}")
                nc.scalar.activation(out=wt[:, off:THALF], in_=sc[:, off:THALF],
                                     func=EXP, scale=SCALE)
                if pend is not None:
                    emit_av(pend)
                if mid is not None and j in mid:
                    mid[j]()
                pend = (j, pieces, wt)
            emit_av(pend)
            # av row 0 is the softmax denominator (ones-column of v): the
            # custom-DVE reciprocal reads it straight from PSUM partition 0.
            zr = norm_pool.tile([1, THALF], f32, tag="zr", name=f"zrr{h}_{th}")
            nc.vector.reciprocal_approx_fast(out=zr, in_=av[0:1, 0:THALF])
            # evacuate the accumulator in one fast copy (frees the PSUM
            # slot for the next unit), then normalize u/Z off-path from SBUF
            avc = norm_pool.tile([D + 1, THALF], f32, tag="avc", name=f"avc{h}_{th}")
            nc.vector.tensor_copy(out=avc, in_=av[0:D + 1, 0:THALF])

            # broadcast 1/Z + stage on the GpSimd queue: it is otherwise idle
            # (collective triggers only), stalls there are harmless, and the
            # Sync/Scalar HWDGE groups stay free of late-produced DMAs whose
            # queue-counter semaphores would poison later consumers.
            zb = norm_pool.tile([D + 1, THALF], f32, tag="zb", name=f"zb{h}_{th}")
            zr_b = bass.AP(tensor=zr.tensor, offset=zr.offset,
                           ap=[list(zr.ap[0]), [0, D + 1], [1, THALF]])
            nc.gpsimd.dma_start(out=zb, in_=zr_b)
            # multiply rows 0:65 (partition-0-aligned for the DVE); row 0 is
            # Z * 1/Z and is simply not staged
            stage = norm_pool.tile([D + 1, THALF], b16, tag="stage", name=f"st{h}_{th}")
            nc.vector.tensor_mul(out=stage, in0=avc, in1=zb)
            nc.gpsimd.dma_start(out=cc_in[th][64 * h:64 * (h + 1), :],
                                in_=stage[1:D + 1, :])

        RG = [[0, 1], [2, 3], [4, 5], [6, 7]]

        def allgather(th, p):
          # head pair {2p, 2p+1} of token-half th -> ci-tiles p (rank0) and
          # 4+p (rank1), each complete
          with nc.named_scope(f"ag{th}_{p}"):
            import concourse.mybir as mybir_mod
            nc.gpsimd.collective_compute(
                "AllGather", mybir_mod.AluOpType.bypass, replica_groups=RG,
                ins=[cc_in[th][128 * p:128 * (p + 1), :].opt()],
                outs=[cc_out[th][p].opt()])
            cc_r = cc_out[th][p].rearrange("(ci p2) t -> ci p2 t", p2=P)
            nc.sync.dma_start(out=ccout_sb[:, th, p, :], in_=cc_r[0])
            nc.sync.dma_start(out=ccout_sb[:, th, 4 + p, :], in_=cc_r[1])

        def allgather_single(h):
          # single head h of token-half 1 -> 64-row halves of ci-tiles h//2
          # (rank0) and 4 + h//2 (rank1); keeps the tail AllGather small
          with nc.named_scope(f"ag1s_{h}"):
            import concourse.mybir as mybir_mod
            nc.gpsimd.collective_compute(
                "AllGather", mybir_mod.AluOpType.bypass, replica_groups=RG,
                ins=[cc_in[1][64 * h:64 * (h + 1), :].opt()],
                outs=[cc_out1s[h - 6].opt()])
            r0 = 64 * (h % 2)
            nc.sync.dma_start(out=ccout_sb[r0:r0 + 64, 1, h // 2, :],
                              in_=cc_out1s[h - 6][0:64, :])
            nc.sync.dma_start(out=ccout_sb[r0:r0 + 64, 1, 4 + h // 2, :],
                              in_=cc_out1s[h - 6][64:128, :])

        def ffn_tile0(tt):
          # full single-pass FFN tile for token-half 0 (all AGs landed)
          with nc.named_scope("ffn"):
            ps = flex_pool.tile([P, COH], f32, tag="flex", name=f"fps{tt}")
            for k, ci in enumerate((0, 4, 1, 5, 2, 6, 3, 7)):
                nc.tensor.matmul(
                    ps, lhsT=ccout_sb[:, 0, ci, P * tt:P * (tt + 1)],
                    rhs=wfT_sb[:, ci, :],
                    start=(k == 0), stop=(k == NCT - 1))
            ysb = y_pool.tile([P, COH], b16, tag="y", name=f"y{tt}")
            nc.vector.tensor_add(out=ysb, in0=ps, in1=biasb_sb)
            nc.vector.tensor_scalar_max(ysb, ysb, 0.0)
            nc.sync.dma_start(out=y.rearrange("(tt p) co -> tt p co", p=P)[tt],
                              in_=ysb)

        # ---- filler queues: projection/FFN chunks drained into the
        # attention j-loops to keep TensorE dense (and the HAM gate warm).
        # QA runs during token-half 0 (everything th1 units need up front);
        # QB runs inside the th1 units, sized to their exp-paced slack.
        fillQA = deque()   # th0-phase fillers (input-DMA gated only)
        for hp in (1, 2, 3):
            for i in (0, 4, 1, 5):
                fillQA.append(lambda hp=hp, i=i: qk_chunk(hp, i))
        for st in range(8, NTT):
            fillQA.append(lambda st=st: v_proj(st))
        for i in (2, 6, 3, 7):
            fillQA.append(lambda i=i: qk_chunk(0, i))

        def popA():
            if fillQA:
                fillQA.popleft()()

        # ---- emission order --------------------------------------------------
        # upfront: q/k for head-pair 0 over tokens 0:1024, v tiles 0:4
        for i in (0, 4, 1, 5):
            qk_chunk(0, i)
        for st in range(4):
            v_proj(st)
        # token-half 0 attention; v st4-7 finish inside unit 0
        attn_unit(0, 0, mid={1: lambda: v_proj(4), 2: lambda: v_proj(5),
                             3: lambda: v_proj(6), 4: lambda: v_proj(7),
                             5: popA, 7: popA})
        popA()
        for h in range(1, HPC):
            attn_unit(h, 0, mid={1: popA, 3: popA, 5: popA})
            if h % 2 == 1:
                allgather(0, h // 2)
            popA()
        while fillQA:  # all projection work must land before token-half 1
            popA()

        # token-half 1 attention: qk g23 chunks (units 0-3, meeting their
        # per-head-pair deadlines) and th0 FFN tiles (units 4-7) are
        # hard-assigned to the units' exp-paced slack so the PE never idles
        # long enough to trip the HAM clock gate.
        def qf(hp, i):
            return lambda: qk_chunk(hp, i)

        def ff(tt):
            return lambda: ffn_tile0(tt)

        th1_mids = [
            {2: qf(1, 2), 6: qf(1, 6), 10: qf(1, 3)},
            {1: qf(1, 7), 5: qf(2, 2), 9: qf(2, 6)},
            {1: qf(2, 3), 5: qf(2, 7), 9: qf(3, 2)},
            {1: qf(3, 6), 5: qf(3, 3), 9: qf(3, 7)},
            {1: ff(0), 5: ff(1), 10: ff(2)},
            {1: ff(3), 7: ff(4)},
            {1: ff(5), 7: ff(6)},
            {2: ff(7), 8: popA},
        ]
        attn_unit(0, 1, mid=th1_mids[0])
        for h in range(1, HPC):
            attn_unit(h, 1, mid=th1_mids[h])
            if h % 2 == 1 and h < 5:
                allgather(1, h // 2)
            if h == 5:
                allgather(1, 2)
            if h == 6:
                allgather_single(6)
        allgather_single(7)
        while fillQA:
            popA()

        # ---- token-half 1 FFN in phases: ci{0,1,4,5} (pairs 0,1 landed long
        # ago), ci{2,6} (pair 2), the head-6 halves of ci{3,7} (single AG 6),
        # and only the head-7 halves (16 K=64 matmuls) wait on the last AG.
        with nc.named_scope("ffn1"):
            ftiles = []
            for bi in range(2):
                buf = sc_pool.tile([P, 2 * COH], f32, tag="sc", name=f"fpsc{bi}")
                ftiles += [buf[:, 0:COH], buf[:, COH:2 * COH]]
            buf = av_pool.tile([P, 2 * COH], f32, tag="av", name="fpav")
            ftiles += [buf[:, 0:COH], buf[:, COH:2 * COH]]
            ftiles += [flex_pool.tile([P, COH], f32, tag="flex", name=f"fpfx{i}")
                       for i in range(2)]
            for phase in ((0, 4, 1, 5), (2, 6)):
                for tl in range(8):
                    for ci in phase:
                        nc.tensor.matmul(
                            ftiles[tl], lhsT=ccout_sb[:, 1, ci, P * tl:P * (tl + 1)],
                            rhs=wfT_sb[:, ci, :],
                            start=(ci == 0), stop=False)
            for rows in (slice(0, 64), slice(64, 128)):   # head 6, then head 7
                for tl in range(8):
                    for ci in (3, 7):
                        nc.tensor.matmul(
                            ftiles[tl],
                            lhsT=ccout_sb[rows, 1, ci, P * tl:P * (tl + 1)],
                            rhs=wfT_sb[rows, ci, :],
                            start=False,
                            stop=(rows.start == 64 and ci == 7))
                    if rows.start == 64:
                        ysb = y_pool.tile([P, COH], b16, tag="y", name=f"y1_{tl}")
                        nc.vector.tensor_add(out=ysb, in0=ftiles[tl],
                                             in1=biasb_sb)
                        nc.vector.tensor_scalar_max(ysb, ysb, 0.0)
                        nc.sync.dma_start(
                            out=y.rearrange("(tt p) co -> tt p co", p=P)[8 + tl],
                            in_=ysb)

    nc.compile()
    return nc


def _pack(a):
    """[C, M] -> [P, NCT*M]: row ct*128+p, col m  ->  row p, col ct*M+m.
    Gives every per-partition DMA line NCT*M contiguous elements."""
    Cc, M = a.shape
    return np.ascontiguousarray(
        a.reshape(NCT, P, M).transpose(1, 0, 2).reshape(P, NCT * M))


def make_in_maps(x, Wq, Wk, Wv, Wf, bf):
    x = np.asarray(x, np.float32)
    mey_m = np.ascontiguousarray(-10000.0 * np.eye(P, dtype=np.float32)).astype(bf16)
    mls_m = np.ascontiguousarray(
        np.tril(np.ones((P, P), np.float32), -1)).astype(bf16)
    bf_f = np.asarray(bf, np.float32)
    wfT_f = np.asarray(Wf, np.float32).T
    in_maps = []
    for core in range(8):
        b, p = divmod(core, 2)
        sl = slice(HPC * p, HPC * (p + 1))
        xT_f = x[b].T  # [C, T]
        m = {
            "wq": _pack(np.asarray(Wq, np.float32)[:, sl].reshape(C, HPC * D)
                        ).astype(bf16),
            "wk": _pack(np.asarray(Wk, np.float32)[:, sl].reshape(C, HPC * D)
                        ).astype(bf16),
            "wv": _pack(np.asarray(Wv, np.float32)[:, sl].reshape(C, HPC * D)
                        ).astype(bf16),
            "wfT": _pack(wfT_f[:, COH * p:COH * (p + 1)]).astype(bf16),
            "mey": mey_m,
            "mls": mls_m,
            "biasb": np.ascontiguousarray(np.tile(
                bf_f[None, COH * p:COH * (p + 1)], (P, 1))),
        }
        for g in range(4):
            m[f"xg{g}"] = _pack(xT_f[:, 512 * g:512 * (g + 1)]).astype(bf16)
        in_maps.append(m)
    return in_maps


def run(x, Wq, Wk, Wv, Wf, bf, trace=False, **spmd_kwargs):
    from concourse.bass_utils import run_bass_kernel_spmd

    if "nc" not in _CACHE:
        _CACHE["nc"] = build_nc()
    nc = _CACHE["nc"]
    in_maps = make_in_maps(x, Wq, Wk, Wv, Wf, bf)
    res = run_bass_kernel_spmd(
        nc, in_maps, core_ids=list(range(8)), trace=trace, **spmd_kwargs)
    out = np.zeros((B, T, C), np.float32)
    for core in range(8):
        b, p = divmod(core, 2)
        out[b, :, COH * p:COH * (p + 1)] = \
            np.asarray(res.results[core]["y"]).astype(np.float32)
    return out, res


def kernel(x, Wq, Wk, Wv, Wf, bf):
    out, _ = run(x, Wq, Wk, Wv, Wf, bf, trace=False)
    return out
